# revision 43
# baseline (speedup 1.0000x reference)
"""Trainium2 Bass kernel for nn_MultiHeadSelfAttention_55654186222044.

Reference math (per batch b, per "slice" h of the reshaped activations):
    xs  = x[b,:,h*64:(h+1)*64]                  (T=1024, D=64)
    q_i = xs @ Wq[i].T + bq[i]   (per param set i=0..15), same k_i, v_i
    scores_i = q_i.T @ k_i / 8   (64x64, contraction over T!)
    w_i = softmax(scores_i, axis=-1)
    o_i = v_i @ w_i.T ;  cat = concat_i o_i     (T, 1024)
    out[b,h] = cat @ Wf.T + bf                  (T, 1024)

Because attention is over the feature dim, everything collapses through a
65x65 Gram matrix G = xa.T @ xa (xa = [xs, 1]):
    P         = G @ W~k_all                       (65, 1024)
    scT chunk = P_chunk.T @ W~q chunk  -> diagonal 64x64 blocks are
                scores_i^T (softmax axis lands on the psum partition dim)
    M~_i      = exp(scT_i).T @ [Wv_aug_i | bv | 1] (last col = denominator)
    M_i       = M~_i * (1/denom) per row
    N         = M.T @ Wf.T + u64 x bf             (65, 1024)
    out[b,h]  = xa @ N
This cuts FLOPs ~10x vs the naive dataflow. |scores| < ~50 so exp needs no
max-subtraction (f32 psum, bf16 storage). Output is written fp16 (error
budget is ~12x what fp16 costs).

Sharding: 32 independent (b, h) slices; 8 cores x 4 slices. Core c takes
b = c//4 and heads 4*(c%4)..4*(c%4)+3 so its x columns are contiguous.
Weights replicated, no collectives.

Two hardware quirks dominate the schedule:
 1. Each dma_start costs ~0.7us of SERIAL issue time on its queue engine,
    so inputs arrive as 7 packed blobs (first-use order; wft split into
    nh0/nh1 column halves so the first N-stage only waits for the first),
    alternating sync/gpsimd queues; output DMAs issue from gpsimd.
 2. The HAM clock gate: PE default is 1.2 GHz; only ~3.4us of sustained
    gapless matmul streaming lifts it to 2.4 GHz and idle re-throttles.
    A ~20-matmul dummy warmup streams from t=0 so the clock is up and the
    activity window stays warm until the input DMAs land; after that the
    ladder (each tail striped with the next head) keeps the PE stream
    dense. Extra woven fillers measured as a net loss (they lengthen the
    stream without flipping the gate earlier), so only the warmup remains.
"""

import numpy as np
import ml_dtypes

B, T, E, H = 2, 1024, 1024, 16
D = E // H
SCALE = float(np.sqrt(D))
NCORES = 8

_CACHE = {}


def _build_nc():
    from contextlib import ExitStack

    import concourse.bass as bass
    import concourse.mybir as mybir
    import concourse.tile as tile
    from concourse import bacc

    dt = mybir.dt
    AF = mybir.ActivationFunctionType
    MUL = mybir.AluOpType.mult

    nc = bacc.Bacc(None)
    xhb_d = nc.declare_dram_parameter("xhb", [128, 2080], dt.float16, False)
    wqk_d = nc.declare_dram_parameter("wqk", [65, 2048], dt.float16, False)
    bfh_d = nc.declare_dram_parameter("bfh", [1, 1024], dt.float16, False)
    wva_d = nc.declare_dram_parameter("wva", [128, 1056], dt.bfloat16, False)
    wfta_d = nc.declare_dram_parameter("wfta", [128, 4096], dt.float16, False)
    wftb_d = nc.declare_dram_parameter("wftb", [128, 4096], dt.float16, False)
    xt0_d = nc.declare_dram_parameter("xt0", [65, 1024], dt.float16, False)
    xtr_d = nc.declare_dram_parameter("xtr", [65, 3072], dt.float16, False)
    out_d = nc.declare_dram_parameter("out", [4, 1024, 1024], dt.float16, True)

    with ExitStack() as ctx:
        tc = ctx.enter_context(tile.TileContext(nc))
        consts = ctx.enter_context(tc.tile_pool(name="consts", bufs=1))
        sbp = ctx.enter_context(tc.tile_pool(name="sbp", bufs=3))
        outp = ctx.enter_context(tc.tile_pool(name="outp", bufs=10))
        ps_w = ctx.enter_context(tc.tile_pool(name="ps_w", bufs=2, space="PSUM"))
        ps_o = ctx.enter_context(tc.tile_pool(name="ps_o", bufs=3, space="PSUM"))
        ps_sc = ctx.enter_context(tc.tile_pool(name="ps_sc", bufs=1, space="PSUM"))
        ps_sm = ctx.enter_context(tc.tile_pool(name="ps_sm", bufs=2, space="PSUM"))

        wqk = consts.tile([65, 2048], dt.float16, name="wqk")
        nc.gpsimd.dma_start(out=wqk[:], in_=wqk_d[:, :])
        xhb = consts.tile([128, 2080], dt.float16, name="xhb")
        nc.sync.dma_start(out=xhb[:], in_=xhb_d[:, :])
        wva = consts.tile([128, 1056], dt.bfloat16, name="wva")
        nc.gpsimd.dma_start(out=wva[:], in_=wva_d[:, :])
        wfta = consts.tile([128, 4096], dt.float16, name="wfta")
        nc.sync.dma_start(out=wfta[:], in_=wfta_d[:, :])
        wftb = consts.tile([128, 4096], dt.float16, name="wftb")
        nc.gpsimd.dma_start(out=wftb[:], in_=wftb_d[:, :])
        xt0 = consts.tile([65, 1024], dt.float16, name="xt0")
        nc.sync.dma_start(out=xt0[:], in_=xt0_d[:, :])
        bfh = consts.tile([1, 1024], dt.float16, name="bfh")
        nc.gpsimd.dma_start(out=bfh[:], in_=bfh_d[:, :])
        xtr = consts.tile([65, 3072], dt.float16, name="xtr")
        nc.sync.dma_start(out=xtr[:], in_=xtr_d[:, :])

        def xh_ap(j, c):
            return xhb[:, j * 520 + c * 65 : j * 520 + (c + 1) * 65]

        def wkt_ap(s, e):
            return wqk[:, s:e]

        def wqt_ap(s, e):
            return wqk[:, 1024 + s : 1024 + e]

        def wva_ap(i):
            # head i's [Wv | bv | 1] block, rows 0:64 duplicated at 64:128
            return wva[:, i * 66 : (i + 1) * 66]

        def wft_ap(c, nh):
            # wft split by output-column half: nh0 half in wfta, nh1 in wftb
            blob = wfta if nh == 0 else wftb
            return blob[:, c * 512 : (c + 1) * 512]

        def xt_ap(j, s, e):
            if j == 0:
                return xt0[:, s:e]
            return xtr[:, (j - 1) * 1024 + s : (j - 1) * 1024 + e]

        ub = consts.tile([1, 65], dt.float16, name="ub")
        nc.vector.memset(ub[:], 0.0)
        nc.vector.memset(ub[:, 64:65], 1.0)

        warm = consts.tile([128, 512], dt.float16, name="warm")
        nc.vector.memset(warm[:], 0.0)

        _fc = [0]

        def filler(n, cols):
            """Dummy matmuls that keep the HAM activity window busy through
            sparse phases. Each allocates a fresh pso-tag tile (single-matmul
            group) so it composes safely with P/out stages and never with an
            open accumulation."""
            for _ in range(n):
                _fc[0] += 1
                fps = ps_o.tile(
                    [128, 512], dt.float32, name=f"fill_{_fc[0]}", tag="pso"
                )
                nc.tensor.matmul(
                    fps[:, 0:cols], warm[:, 0:128], warm[:, 0:cols],
                    start=True, stop=True,
                )
                yield

        gsb = {}
        psb = {}
        expC = {}
        rec = {}
        msb = {}
        nsb = {}

        def emit_gp(*js):
            """G and P stages for the given slices."""
            for j in js:
                gps = ps_sm.tile([65, 65], dt.float32, name=f"gps_{j}", tag="pssm")
                for c in range(8):
                    nc.tensor.matmul(
                        gps[:], xh_ap(j, c), xh_ap(j, c),
                        start=(c == 0), stop=(c == 7),
                    )
                    if c == 3:
                        yield
                gsb[j] = sbp.tile([65, 65], dt.float16, name=f"gsb_{j}", tag="gsb")
                nc.vector.tensor_copy(out=gsb[j][:], in_=gps[:])
                yield
            for j in js:
                psb[j] = sbp.tile([65, 1024], dt.float16, name=f"psb_{j}", tag="psb")
                for nh in range(2):
                    pps = ps_o.tile([65, 512], dt.float32, name=f"pps_{j}_{nh}", tag="pso")
                    nc.tensor.matmul(
                        pps[:], gsb[j][:], wkt_ap(nh * 512, (nh + 1) * 512),
                        start=True, stop=True,
                    )
                    if nh == 0:
                        nc.vector.tensor_copy(out=psb[j][:, 0:512], in_=pps[:])
                    else:
                        nc.scalar.copy(out=psb[j][:, 512:1024], in_=pps[:])
                    yield

        def emit_scm(*js):
            """scoresT+exp and M stages for the given slices."""
            for j in js:
                # scT chunks: diag 64x64 blocks of P_chunk.T @ W~q_chunk
                expC[j] = sbp.tile([128, 8, 128], dt.bfloat16, name=f"expC_{j}", tag="expC")
                for t in range(2):
                    scp = ps_sc.tile([128, 512], dt.float32, name=f"scp_{j}_{t}", tag="pssc")
                    for u in range(4):
                        c = 4 * t + u
                        nc.tensor.matmul(
                            scp[:, u * 128 : (u + 1) * 128],
                            psb[j][:, c * 128 : (c + 1) * 128],
                            wqt_ap(c * 128, (c + 1) * 128),
                            start=True, stop=True,
                        )
                        if u == 1:
                            yield
                    nc.scalar.activation(
                        out=expC[j][:, 4 * t : 4 * t + 4, :], in_=scp[:], func=AF.Exp
                    )
                    yield
            for j in js:
                rec[j] = sbp.tile([128, 8], dt.float32, name=f"rec_{j}", tag="rec")
                msb[j] = sbp.tile([128, 8, 65], dt.float16, name=f"msb_{j}", tag="msb")
                for t in range(2):
                    mp4 = ps_sm.tile([128, 4, 66], dt.float32, name=f"mp4_{j}_{t}", tag="pssm")
                    for u in range(4):
                        c = 4 * t + u
                        nc.tensor.matmul(
                            mp4[0:64, u, :], expC[j][0:64, c, 0:64], wva_ap(2 * c)[0:64, :],
                            start=True, stop=True,
                        )
                        nc.tensor.matmul(
                            mp4[64:128, u, :], expC[j][64:128, c, 64:128],
                            wva_ap(2 * c + 1)[64:128, :],
                            start=True, stop=True,
                        )
                        if u == 1:
                            yield
                    r4 = rec[j][:, 4 * t : 4 * t + 4]
                    nc.vector.reciprocal(out=r4.unsqueeze(-1), in_=mp4[:, :, 65:66])
                    nc.vector.scalar_tensor_tensor(
                        out=msb[j][:, 4 * t : 4 * t + 4, :],
                        in0=mp4[:, :, 0:65],
                        scalar=1.0,
                        in1=r4.unsqueeze(-1).broadcast_to([128, 4, 65]),
                        op0=MUL,
                        op1=MUL,
                    )
                    yield

        def emit_head(*js):
            for j in js:
                yield from emit_gp(j)
                yield from emit_scm(j)

        def emit_tail(*js):
            """N and out stages for the given slices."""
            for j in js:
                nsb[j] = sbp.tile([65, 1024], dt.float16, name=f"nsb_{j}", tag="nsb")
                for nh in range(2):
                    nsp = ps_w.tile([65, 512], dt.float32, name=f"nsp_{j}_{nh}", tag="psw")
                    for c in range(8):
                        nc.tensor.matmul(
                            nsp[:], msb[j][:, c, :], wft_ap(c, nh),
                            start=(c == 0), stop=False,
                        )
                        if c % 2 == 1:
                            yield
                    nc.tensor.matmul(
                        nsp[:], ub[:], bfh[:, nh * 512 : (nh + 1) * 512],
                        start=False, stop=True,
                    )
                    if nh == 0:
                        nc.vector.tensor_copy(out=nsb[j][:, 0:512], in_=nsp[:])
                    else:
                        nc.scalar.copy(out=nsb[j][:, 512:1024], in_=nsp[:])
                    yield
            for j in js:
                for c in range(8):
                    osb = outp.tile([128, 1024], dt.float16, name=f"osb_{j}_{c}", tag="osb")
                    for nh in range(2):
                        ops = ps_o.tile([128, 512], dt.float32, name=f"ops_{j}_{c}_{nh}", tag="pso")
                        nc.tensor.matmul(
                            ops[:], xt_ap(j, c * 128, (c + 1) * 128),
                            nsb[j][:, nh * 512 : (nh + 1) * 512],
                            start=True, stop=True,
                        )
                        if (c + nh) % 2 == 0:
                            nc.vector.tensor_copy(
                                out=osb[:, nh * 512 : (nh + 1) * 512], in_=ops[:]
                            )
                        else:
                            nc.scalar.copy(
                                out=osb[:, nh * 512 : (nh + 1) * 512], in_=ops[:]
                            )
                        yield
                    nc.gpsimd.dma_start(
                        out=out_d[j, c * 128 : (c + 1) * 128, :], in_=osb[:]
                    )

        def drain(gen):
            for _ in gen:
                pass

        def stripe(a, b):
            a_live, b_live = True, True
            while a_live or b_live:
                if a_live:
                    a_live = next(a, _SENT) is not _SENT
                if b_live:
                    b_live = next(b, _SENT) is not _SENT

        # v1 ladder: warmup flips HAM while inputs land; each tail is striped
        # with the next head so the middle stays PE-bound. (Measured: extra
        # woven fillers do NOT un-throttle earlier — they only lengthen the
        # stream — so only the initial warmup remains.)
        drain(filler(20, 512))
        drain(emit_head(0))
        stripe(emit_head(1), emit_tail(0))
        stripe(emit_tail(1), emit_head(2))
        stripe(emit_tail(2), emit_head(3))
        drain(emit_tail(3))

    nc.finalize()
    return nc


_SENT = object()


def _prep_weights(Wq, bq, Wk, bk, Wv, bv, Wf, bf):
    wqk = np.zeros((65, 2048), np.float16)
    wqk[:64, 0:1024] = np.transpose(Wk, (2, 0, 1)).reshape(64, H * D).astype(np.float16)
    wqk[64, 0:1024] = bk.reshape(H * D).astype(np.float16)
    wqk[:64, 1024:2048] = (
        np.transpose(Wq, (2, 0, 1)).reshape(64, H * D) / SCALE
    ).astype(np.float16)
    wqk[64, 1024:2048] = (bq.reshape(H * D) / SCALE).astype(np.float16)
    bfh = bf.reshape(1, 1024).astype(np.float16)
    wva_h = np.zeros((64, 16, 66), ml_dtypes.bfloat16)
    wva_h[:, :, :64] = np.transpose(Wv, (1, 0, 2)).astype(ml_dtypes.bfloat16)
    wva_h[:, :, 64] = bv.T.astype(ml_dtypes.bfloat16)
    wva_h[:, :, 65] = 1.0
    wva = np.concatenate([wva_h, wva_h], axis=0).reshape(128, 1056)
    wft = np.ascontiguousarray(
        Wf.T.reshape(8, 128, 1024).transpose(1, 0, 2)
    ).astype(np.float16)
    wfta = np.ascontiguousarray(wft[:, :, 0:512].reshape(128, 4096))
    wftb = np.ascontiguousarray(wft[:, :, 512:1024].reshape(128, 4096))
    return wqk, bfh, wva, wfta, wftb


def _prep_x(xs):
    """xs (1024, 256) f32 -> xhb (128, 2080) fp16 (slice-major xa chunks with
    ones col), xt0 (65, 1024) / xtr (65, 3072) fp16 xa^T with ones row."""
    x16 = xs.astype(np.float16)
    xh = np.ones((4, 128, 8, 65), np.float16)
    xh[:, :, :, :64] = x16.reshape(8, 128, 4, 64).transpose(2, 1, 0, 3)
    xhb = np.ascontiguousarray(xh.transpose(1, 0, 2, 3).reshape(128, 2080))
    xt = np.ones((4, 65, 1024), np.float16)
    xt[:, :64, :] = x16.reshape(1024, 4, 64).transpose(1, 2, 0)
    xt0 = np.ascontiguousarray(xt[0])
    xtr = np.ascontiguousarray(xt[1:4].transpose(1, 0, 2).reshape(65, 3072))
    return xhb, xt0, xtr


def _run(inputs, trace=False, tmpdir=None):
    from concourse.bass_utils import run_bass_kernel_spmd

    if "nc" not in _CACHE:
        _CACHE["nc"] = _build_nc()
    nc = _CACHE["nc"]

    x = np.ascontiguousarray(np.asarray(inputs["x"]), dtype=np.float32)
    wqk, bfh, wva, wfta, wftb = _prep_weights(
        *(np.asarray(inputs[k], dtype=np.float32) for k in
          ("Wq", "bq", "Wk", "bk", "Wv", "bv", "Wf", "bf"))
    )
    common = dict(wqk=wqk, bfh=bfh, wva=wva, wfta=wfta, wftb=wftb)
    in_maps = []
    for c in range(NCORES):
        xs = np.ascontiguousarray(x[c // 4][:, (c % 4) * 256 : (c % 4 + 1) * 256])
        xhb, xt0, xtr = _prep_x(xs)
        in_maps.append(dict(xhb=xhb, xt0=xt0, xtr=xtr, **common))

    res = run_bass_kernel_spmd(
        nc, in_maps, list(range(NCORES)), trace=trace, tmpdir=tmpdir
    )
    out = np.empty((B, H, T, E), np.float32)
    for c in range(NCORES):
        out[c // 4, 4 * (c % 4) : 4 * (c % 4) + 4] = res.results[c]["out"].astype(
            np.float32
        )
    return out, res.exec_time_ns


def kernel(**inputs) -> np.ndarray:
    out, _ = _run(inputs, trace=False)
    return out


# revision 44
# speedup vs baseline: 1.0361x; 1.0361x over previous
"""Trainium2 Bass kernel for nn_MultiHeadSelfAttention_55654186222044.

Reference math (per batch b, per "slice" h of the reshaped activations):
    xs  = x[b,:,h*64:(h+1)*64]                  (T=1024, D=64)
    q_i = xs @ Wq[i].T + bq[i]   (per param set i=0..15), same k_i, v_i
    scores_i = q_i.T @ k_i / 8   (64x64, contraction over T!)
    w_i = softmax(scores_i, axis=-1)
    o_i = v_i @ w_i.T ;  cat = concat_i o_i     (T, 1024)
    out[b,h] = cat @ Wf.T + bf                  (T, 1024)

Because attention is over the feature dim, everything collapses through a
65x65 Gram matrix G = xa.T @ xa (xa = [xs, 1]):
    P         = G @ W~k_all                       (65, 1024)
    scT chunk = P_chunk.T @ W~q chunk  -> diagonal 64x64 blocks are
                scores_i^T (softmax axis lands on the psum partition dim)
    M~_i      = exp(scT_i).T @ [Wv_aug_i | bv | 1] (last col = denominator)
    M_i       = M~_i * (1/denom) per row
    N         = M.T @ Wf.T + u64 x bf             (65, 1024)
    out[b,h]  = xa @ N
This cuts FLOPs ~10x vs the naive dataflow. |scores| < ~50 so exp needs no
max-subtraction (f32 psum, bf16 storage). Output is written fp16 (error
budget is ~12x what fp16 costs).

Sharding: 32 independent (b, h) slices; 8 cores x 4 slices. Core c takes
b = c//4 and heads 4*(c%4)..4*(c%4)+3 so its x columns are contiguous.
Weights replicated, no collectives.

Two hardware quirks dominate the schedule:
 1. Each dma_start costs ~0.7us of SERIAL issue time on its queue engine,
    so inputs arrive as 7 packed blobs (first-use order; wft split into
    nh0/nh1 column halves so the first N-stage only waits for the first),
    alternating sync/gpsimd queues; output DMAs issue from gpsimd.
 2. The HAM clock gate: PE default is 1.2 GHz; only ~3.4us of sustained
    gapless matmul streaming lifts it to 2.4 GHz and idle re-throttles.
    A ~20-matmul dummy warmup streams from t=0 so the clock is up and the
    activity window stays warm until the input DMAs land; after that the
    ladder (each tail striped with the next head) keeps the PE stream
    dense. Extra woven fillers measured as a net loss (they lengthen the
    stream without flipping the gate earlier), so only the warmup remains.
"""

import numpy as np
import ml_dtypes

B, T, E, H = 2, 1024, 1024, 16
D = E // H
SCALE = float(np.sqrt(D))
NCORES = 8

_CACHE = {}


def _build_nc():
    from contextlib import ExitStack

    import concourse.bass as bass
    import concourse.mybir as mybir
    import concourse.tile as tile
    from concourse import bacc

    dt = mybir.dt
    AF = mybir.ActivationFunctionType
    MUL = mybir.AluOpType.mult

    nc = bacc.Bacc(None)
    xhb_d = nc.declare_dram_parameter("xhb", [128, 2080], dt.float16, False)
    wqk_d = nc.declare_dram_parameter("wqk", [65, 2048], dt.float16, False)
    bfh_d = nc.declare_dram_parameter("bfh", [1, 1024], dt.float16, False)
    wva_d = nc.declare_dram_parameter("wva", [128, 1056], dt.bfloat16, False)
    wfta_d = nc.declare_dram_parameter("wfta", [128, 4096], dt.float16, False)
    wftb_d = nc.declare_dram_parameter("wftb", [128, 4096], dt.float16, False)
    xt0_d = nc.declare_dram_parameter("xt0", [65, 1024], dt.float16, False)
    xtr_d = nc.declare_dram_parameter("xtr", [65, 3072], dt.float16, False)
    out_d = nc.declare_dram_parameter("out", [4, 1024, 1024], dt.float16, True)

    with ExitStack() as ctx:
        tc = ctx.enter_context(tile.TileContext(nc))
        consts = ctx.enter_context(tc.tile_pool(name="consts", bufs=1))
        sbp = ctx.enter_context(tc.tile_pool(name="sbp", bufs=3))
        outp = ctx.enter_context(tc.tile_pool(name="outp", bufs=10))
        ps_w = ctx.enter_context(tc.tile_pool(name="ps_w", bufs=2, space="PSUM"))
        ps_o = ctx.enter_context(tc.tile_pool(name="ps_o", bufs=3, space="PSUM"))
        ps_sc = ctx.enter_context(tc.tile_pool(name="ps_sc", bufs=1, space="PSUM"))
        ps_sm = ctx.enter_context(tc.tile_pool(name="ps_sm", bufs=2, space="PSUM"))

        wqk = consts.tile([65, 2048], dt.float16, name="wqk")
        nc.gpsimd.dma_start(out=wqk[:], in_=wqk_d[:, :])
        xhb = consts.tile([128, 2080], dt.float16, name="xhb")
        nc.sync.dma_start(out=xhb[:], in_=xhb_d[:, :])
        wva = consts.tile([128, 1056], dt.bfloat16, name="wva")
        nc.gpsimd.dma_start(out=wva[:], in_=wva_d[:, :])
        wfta = consts.tile([128, 4096], dt.float16, name="wfta")
        nc.sync.dma_start(out=wfta[:], in_=wfta_d[:, :])
        wftb = consts.tile([128, 4096], dt.float16, name="wftb")
        nc.gpsimd.dma_start(out=wftb[:], in_=wftb_d[:, :])
        xt0 = consts.tile([65, 1024], dt.float16, name="xt0")
        nc.sync.dma_start(out=xt0[:], in_=xt0_d[:, :])
        bfh = consts.tile([1, 1024], dt.float16, name="bfh")
        nc.gpsimd.dma_start(out=bfh[:], in_=bfh_d[:, :])
        xtr = consts.tile([65, 3072], dt.float16, name="xtr")
        nc.sync.dma_start(out=xtr[:], in_=xtr_d[:, :])

        def xh_ap(j, c):
            return xhb[:, j * 520 + c * 65 : j * 520 + (c + 1) * 65]

        def wkt_ap(s, e):
            return wqk[:, s:e]

        def wqt_ap(s, e):
            return wqk[:, 1024 + s : 1024 + e]

        def wva_ap(i):
            # head i's [Wv | bv | 1] block, rows 0:64 duplicated at 64:128
            return wva[:, i * 66 : (i + 1) * 66]

        def wft_ap(c, nh):
            # wft split by output-column half: nh0 half in wfta, nh1 in wftb
            blob = wfta if nh == 0 else wftb
            return blob[:, c * 512 : (c + 1) * 512]

        def xt_ap(j, s, e):
            if j == 0:
                return xt0[:, s:e]
            return xtr[:, (j - 1) * 1024 + s : (j - 1) * 1024 + e]

        ub = consts.tile([1, 65], dt.float16, name="ub")
        nc.vector.memset(ub[:], 0.0)
        nc.vector.memset(ub[:, 64:65], 1.0)

        warm = consts.tile([128, 512], dt.float16, name="warm")
        nc.vector.memset(warm[:], 0.0)

        _fc = [0]

        def filler(n, cols):
            """Dummy matmuls that keep the HAM activity window busy through
            sparse phases. Each allocates a fresh pso-tag tile (single-matmul
            group) so it composes safely with P/out stages and never with an
            open accumulation."""
            for _ in range(n):
                _fc[0] += 1
                fps = ps_o.tile(
                    [128, 512], dt.float32, name=f"fill_{_fc[0]}", tag="pso"
                )
                nc.tensor.matmul(
                    fps[:, 0:cols], warm[:, 0:128], warm[:, 0:cols],
                    start=True, stop=True,
                )
                yield

        gsb = {}
        psb = {}
        expC = {}
        rec = {}
        msb = {}
        nsb = {}

        def emit_gp(*js):
            """G and P stages for the given slices."""
            for j in js:
                gps = ps_sm.tile([65, 65], dt.float32, name=f"gps_{j}", tag="pssm")
                for c in range(8):
                    nc.tensor.matmul(
                        gps[:], xh_ap(j, c), xh_ap(j, c),
                        start=(c == 0), stop=(c == 7),
                    )
                    if c == 3:
                        yield
                gsb[j] = sbp.tile([65, 65], dt.float16, name=f"gsb_{j}", tag="gsb")
                nc.vector.tensor_copy(out=gsb[j][:], in_=gps[:])
                yield
            for j in js:
                psb[j] = sbp.tile([65, 1024], dt.float16, name=f"psb_{j}", tag="psb")
                for nh in range(2):
                    pps = ps_o.tile([65, 512], dt.float32, name=f"pps_{j}_{nh}", tag="pso")
                    nc.tensor.matmul(
                        pps[:], gsb[j][:], wkt_ap(nh * 512, (nh + 1) * 512),
                        start=True, stop=True,
                    )
                    if nh == 0:
                        nc.vector.tensor_copy(out=psb[j][:, 0:512], in_=pps[:])
                    else:
                        nc.scalar.copy(out=psb[j][:, 512:1024], in_=pps[:])
                    yield

        def emit_scm(*js):
            """scoresT+exp and M stages for the given slices."""
            for j in js:
                # scT chunks: diag 64x64 blocks of P_chunk.T @ W~q_chunk
                expC[j] = sbp.tile([128, 8, 128], dt.bfloat16, name=f"expC_{j}", tag="expC")
                for t in range(2):
                    scp = ps_sc.tile([128, 512], dt.float32, name=f"scp_{j}_{t}", tag="pssc")
                    for u in range(4):
                        c = 4 * t + u
                        nc.tensor.matmul(
                            scp[:, u * 128 : (u + 1) * 128],
                            psb[j][:, c * 128 : (c + 1) * 128],
                            wqt_ap(c * 128, (c + 1) * 128),
                            start=True, stop=True,
                        )
                        if u == 1:
                            yield
                    nc.scalar.activation(
                        out=expC[j][:, 4 * t : 4 * t + 4, :], in_=scp[:], func=AF.Exp
                    )
                    yield
            for j in js:
                rec[j] = sbp.tile([128, 8], dt.float32, name=f"rec_{j}", tag="rec")
                msb[j] = sbp.tile([128, 8, 65], dt.float16, name=f"msb_{j}", tag="msb")
                for t in range(2):
                    mp4 = ps_sm.tile([128, 4, 66], dt.float32, name=f"mp4_{j}_{t}", tag="pssm")
                    for u in range(4):
                        c = 4 * t + u
                        nc.tensor.matmul(
                            mp4[0:64, u, :], expC[j][0:64, c, 0:64], wva_ap(2 * c)[0:64, :],
                            start=True, stop=True,
                        )
                        nc.tensor.matmul(
                            mp4[64:128, u, :], expC[j][64:128, c, 64:128],
                            wva_ap(2 * c + 1)[64:128, :],
                            start=True, stop=True,
                        )
                        if u == 1:
                            yield
                    r4 = rec[j][:, 4 * t : 4 * t + 4]
                    nc.vector.reciprocal(out=r4.unsqueeze(-1), in_=mp4[:, :, 65:66])
                    nc.vector.scalar_tensor_tensor(
                        out=msb[j][:, 4 * t : 4 * t + 4, :],
                        in0=mp4[:, :, 0:65],
                        scalar=1.0,
                        in1=r4.unsqueeze(-1).broadcast_to([128, 4, 65]),
                        op0=MUL,
                        op1=MUL,
                    )
                    yield

        def emit_head(*js):
            for j in js:
                yield from emit_gp(j)
                yield from emit_scm(j)

        def emit_tail(*js):
            """N and out stages for the given slices."""
            for j in js:
                nsb[j] = sbp.tile([65, 1024], dt.float16, name=f"nsb_{j}", tag="nsb")
                for nh in range(2):
                    nsp = ps_w.tile([65, 512], dt.float32, name=f"nsp_{j}_{nh}", tag="psw")
                    for c in range(8):
                        nc.tensor.matmul(
                            nsp[:], msb[j][:, c, :], wft_ap(c, nh),
                            start=(c == 0), stop=False,
                        )
                        if c % 2 == 1:
                            yield
                    nc.tensor.matmul(
                        nsp[:], ub[:], bfh[:, nh * 512 : (nh + 1) * 512],
                        start=False, stop=True,
                    )
                    if nh == 0:
                        nc.vector.tensor_copy(out=nsb[j][:, 0:512], in_=nsp[:])
                    else:
                        nc.scalar.copy(out=nsb[j][:, 512:1024], in_=nsp[:])
                    yield
            for j in js:
                for c in range(8):
                    osb = outp.tile([128, 1024], dt.float16, name=f"osb_{j}_{c}", tag="osb")
                    for nh in range(2):
                        ops = ps_o.tile([128, 512], dt.float32, name=f"ops_{j}_{c}_{nh}", tag="pso")
                        nc.tensor.matmul(
                            ops[:], xt_ap(j, c * 128, (c + 1) * 128),
                            nsb[j][:, nh * 512 : (nh + 1) * 512],
                            start=True, stop=True,
                        )
                        if (c + nh) % 2 == 0:
                            nc.vector.tensor_copy(
                                out=osb[:, nh * 512 : (nh + 1) * 512], in_=ops[:]
                            )
                        else:
                            nc.scalar.copy(
                                out=osb[:, nh * 512 : (nh + 1) * 512], in_=ops[:]
                            )
                        yield
                    qeng = nc.gpsimd if c % 2 == 0 else nc.sync
                    qeng.dma_start(
                        out=out_d[j, c * 128 : (c + 1) * 128, :], in_=osb[:]
                    )

        def drain(gen):
            for _ in gen:
                pass

        def stripe(a, b):
            a_live, b_live = True, True
            while a_live or b_live:
                if a_live:
                    a_live = next(a, _SENT) is not _SENT
                if b_live:
                    b_live = next(b, _SENT) is not _SENT

        # v1 ladder: warmup flips HAM while inputs land; each tail is striped
        # with the next head so the middle stays PE-bound. (Measured: extra
        # woven fillers do NOT un-throttle earlier — they only lengthen the
        # stream — so only the initial warmup remains.)
        drain(filler(20, 512))
        drain(emit_head(0))
        stripe(emit_head(1), emit_tail(0))
        stripe(emit_tail(1), emit_head(2))
        stripe(emit_tail(2), emit_head(3))
        drain(emit_tail(3))

    nc.finalize()
    return nc


_SENT = object()


def _prep_weights(Wq, bq, Wk, bk, Wv, bv, Wf, bf):
    wqk = np.zeros((65, 2048), np.float16)
    wqk[:64, 0:1024] = np.transpose(Wk, (2, 0, 1)).reshape(64, H * D).astype(np.float16)
    wqk[64, 0:1024] = bk.reshape(H * D).astype(np.float16)
    wqk[:64, 1024:2048] = (
        np.transpose(Wq, (2, 0, 1)).reshape(64, H * D) / SCALE
    ).astype(np.float16)
    wqk[64, 1024:2048] = (bq.reshape(H * D) / SCALE).astype(np.float16)
    bfh = bf.reshape(1, 1024).astype(np.float16)
    wva_h = np.zeros((64, 16, 66), ml_dtypes.bfloat16)
    wva_h[:, :, :64] = np.transpose(Wv, (1, 0, 2)).astype(ml_dtypes.bfloat16)
    wva_h[:, :, 64] = bv.T.astype(ml_dtypes.bfloat16)
    wva_h[:, :, 65] = 1.0
    wva = np.concatenate([wva_h, wva_h], axis=0).reshape(128, 1056)
    wft = np.ascontiguousarray(
        Wf.T.reshape(8, 128, 1024).transpose(1, 0, 2)
    ).astype(np.float16)
    wfta = np.ascontiguousarray(wft[:, :, 0:512].reshape(128, 4096))
    wftb = np.ascontiguousarray(wft[:, :, 512:1024].reshape(128, 4096))
    return wqk, bfh, wva, wfta, wftb


def _prep_x(xs):
    """xs (1024, 256) f32 -> xhb (128, 2080) fp16 (slice-major xa chunks with
    ones col), xt0 (65, 1024) / xtr (65, 3072) fp16 xa^T with ones row."""
    x16 = xs.astype(np.float16)
    xh = np.ones((4, 128, 8, 65), np.float16)
    xh[:, :, :, :64] = x16.reshape(8, 128, 4, 64).transpose(2, 1, 0, 3)
    xhb = np.ascontiguousarray(xh.transpose(1, 0, 2, 3).reshape(128, 2080))
    xt = np.ones((4, 65, 1024), np.float16)
    xt[:, :64, :] = x16.reshape(1024, 4, 64).transpose(1, 2, 0)
    xt0 = np.ascontiguousarray(xt[0])
    xtr = np.ascontiguousarray(xt[1:4].transpose(1, 0, 2).reshape(65, 3072))
    return xhb, xt0, xtr


def _run(inputs, trace=False, tmpdir=None):
    from concourse.bass_utils import run_bass_kernel_spmd

    if "nc" not in _CACHE:
        _CACHE["nc"] = _build_nc()
    nc = _CACHE["nc"]

    x = np.ascontiguousarray(np.asarray(inputs["x"]), dtype=np.float32)
    wqk, bfh, wva, wfta, wftb = _prep_weights(
        *(np.asarray(inputs[k], dtype=np.float32) for k in
          ("Wq", "bq", "Wk", "bk", "Wv", "bv", "Wf", "bf"))
    )
    common = dict(wqk=wqk, bfh=bfh, wva=wva, wfta=wfta, wftb=wftb)
    in_maps = []
    for c in range(NCORES):
        xs = np.ascontiguousarray(x[c // 4][:, (c % 4) * 256 : (c % 4 + 1) * 256])
        xhb, xt0, xtr = _prep_x(xs)
        in_maps.append(dict(xhb=xhb, xt0=xt0, xtr=xtr, **common))

    res = run_bass_kernel_spmd(
        nc, in_maps, list(range(NCORES)), trace=trace, tmpdir=tmpdir
    )
    out = np.empty((B, H, T, E), np.float32)
    for c in range(NCORES):
        out[c // 4, 4 * (c % 4) : 4 * (c % 4) + 4] = res.results[c]["out"].astype(
            np.float32
        )
    return out, res.exec_time_ns


def kernel(**inputs) -> np.ndarray:
    out, _ = _run(inputs, trace=False)
    return out
